# revision 11
# baseline (speedup 1.0000x reference)
"""Per-pixel predicted 5x5 conv (KPN) on 8 trn2 cores.

Deep-contraction im2col: each 8x4 output tile (one window) is ONE PE matmul
contracting over the 12x8 input patch (96 partitions):

  out[c, t] = sum_p  S[p, c] * M[p, t]
  S[p=(dh,du), c]       = feat[8wh+dh-2, 4ww+du-2, c]   (host-gathered slabs)
  M[p=(dh,du), t=(th,tw)] = kernel[8wh+th, 4ww+tw, (dh-th)*5+(du-tw)]
                            if both tap offsets in [0,5) else 0  (banded)

All 25 taps of 32 output pixels x 32 channels finish in one 32-column
matmul (bf16 in, fp32 PSUM). ACT evacuates PSUM with the bias add fused;
output leaves as bf16. DMA per core: S 6.3MB + M 6.3MB + out 2.1MB.
"""

import sys

for p in ("/opt/pypackages", "/opt/trn_rl_repo"):
    if p not in sys.path:
        sys.path.insert(0, p)

import numpy as np
import ml_dtypes

import concourse.mybir as mybir
from concourse import bacc, tile
from concourse.bass_utils import run_bass_kernel_spmd

B, H, W, C, KK, K = 4, 256, 256, 32, 25, 5
HS = H // 2            # 128 output rows per core
TH, TW = 8, 4          # output tile per window
PH, PU = TH + 4, TW + 4    # input patch dims -> 12*8 = 96 partitions
NP = PH * PU           # 96
NWH, NWW = HS // TH, W // TW   # 16 x 64 = 1024 windows per core
NT = TH * TW           # 32 moving columns per window
SLOTS = 15             # windows of one rho-quadrant per PSUM bank (15*32=480)
NG = (NWH * NWW // 4 + SLOTS - 1) // SLOTS   # 18 evac groups (last partial)
OUTF = (NWH * NWW // 4) * NT  # 8192 free elems per out partition
BF16 = mybir.dt.bfloat16
F32 = mybir.dt.float32

_NC_CACHE = {}


def _build_nc():
    nc = bacc.Bacc(None, target_bir_lowering=False)
    s_d = nc.dram_tensor("s", [NWH // 2, NP, 2 * NWW * C], BF16,
                         kind="ExternalInput")
    m_d = nc.dram_tensor("m", [NWH // 2, NP, 2 * NWW * NT], BF16,
                         kind="ExternalInput")
    bias_d = nc.dram_tensor("biasr", [128, 1], F32, kind="ExternalInput")
    out_d = nc.dram_tensor("out", [128, OUTF], BF16, kind="ExternalOutput")
    GNT = SLOTS * NT                      # 480 out elems per evac group

    with tile.TileContext(nc) as tc:
        with tc.tile_pool(name="const", bufs=1) as cpool, \
             tc.tile_pool(name="sm", bufs=4) as spool, \
             tc.tile_pool(name="osb", bufs=4) as opool, \
             tc.tile_pool(name="psum", bufs=4, space="PSUM") as qpool:
            bias_t = cpool.tile([128, 1], F32, tag="bias")
            nc.sync.dma_start(out=bias_t, in_=bias_d[:, :])

            ps = ob = None
            for wh in range(NWH):
                if wh % 2 == 0:
                    s_t = spool.tile([NP, 2, NWW, C], BF16, tag="s")
                    nc.sync.dma_start(out=s_t, in_=s_d[wh // 2, :, :])
                    m_t = spool.tile([NP, 2, NWW, NT], BF16, tag="m")
                    nc.scalar.dma_start(out=m_t, in_=m_d[wh // 2, :, :])
                for ww in range(NWW):
                    w = wh * NWW + ww        # window index
                    rho, sg = w % 4, w // 4
                    g, slot = sg // SLOTS, sg % SLOTS
                    if rho == 0 and slot == 0:
                        ps = qpool.tile([128, GNT], F32, tag="ps")
                    nc.tensor.matmul(
                        ps[32 * rho:32 * rho + 32,
                           slot * NT:(slot + 1) * NT],
                        s_t[:, wh % 2, ww, :],
                        m_t[:, wh % 2, ww, :],
                        start=True, stop=True,
                        tile_position=(0, 32 * rho))
                    if w == NWH * NWW - 1 or (rho == 3 and slot == SLOTS - 1):
                        nf = (slot + 1) * NT
                        if g % 2 == 0:
                            ob = opool.tile([128, 2 * GNT], BF16, tag="ob")
                        nc.scalar.activation(
                            ob[:, (g % 2) * GNT:(g % 2) * GNT + nf],
                            ps[:, :nf],
                            mybir.ActivationFunctionType.Identity,
                            bias=bias_t[:, :], scale=1.0)
                        if g % 2 == 1 or w == NWH * NWW - 1:
                            g0 = g - (g % 2)
                            tot = (g % 2) * GNT + nf
                            nc.gpsimd.dma_start(
                                out=out_d[:, g0 * GNT:g0 * GNT + tot],
                                in_=ob[:, :tot])
    if not nc.is_finalized():
        nc.finalize()
    return nc


def _get_nc():
    if "nc" not in _NC_CACHE:
        _NC_CACHE["nc"] = _build_nc()
    return _NC_CACHE["nc"]


def _prep_inputs(feat, kernel, bias):
    fb = feat.astype(ml_dtypes.bfloat16)
    kb = kernel.astype(ml_dtypes.bfloat16)
    # broadcastable index arrays over [wh, dh, du, ww, th, tw]
    IH = (8 * np.arange(NWH)[:, None, None, None, None, None]
          + np.arange(TH)[None, None, None, None, :, None]).astype(np.int32)
    IW = (4 * np.arange(NWW)[None, None, None, :, None, None]
          + np.arange(TW)[None, None, None, None, None, :]).astype(np.int32)
    IDI = (np.arange(PH)[None, :, None, None, None, None]
           - np.arange(TH)[None, None, None, None, :, None] + 7).astype(np.int32)
    IDJ = (np.arange(PU)[None, None, :, None, None, None]
           - np.arange(TW)[None, None, None, None, None, :] + 3).astype(np.int32)

    bias_rep = np.ascontiguousarray(
        np.tile(bias.astype(np.float32), 4)[:, None])    # [128,1]

    in_maps = []
    for core in range(8):
        b, hh = core // 2, core % 2
        # padded feat rows [-2, 130) x cols [-2, 258)
        fpad = np.zeros((PH + 8 * (NWH - 1), W + 4, C), ml_dtypes.bfloat16)
        r0 = hh * HS - 2
        lo, hi = max(0, -r0), min(132, H - r0)
        fpad[lo:hi, 2:W + 2] = fb[b, r0 + lo:r0 + hi]
        # S[wh, p=(dh,du), ww, c] = fpad[8wh+dh, 4ww+du, c]
        s_arr = fpad[(8 * np.arange(NWH)[:, None, None, None]
                      + np.arange(PH)[None, :, None, None]),
                     (4 * np.arange(NWW)[None, None, None, :]
                      + np.arange(PU)[None, None, :, None])]  # [wh,dh,du,ww,c]
        s_arr = (s_arr.reshape(NWH // 2, 2, NP, NWW * C)
                 .transpose(0, 2, 1, 3).reshape(NWH // 2, NP, 2 * NWW * C))
        # padded tap table for this core
        kp2 = np.zeros((HS, W, PH + TH - 1, PU + TW - 1), ml_dtypes.bfloat16)
        kc = kb[b, hh * HS:(hh + 1) * HS]                # [128, 256, 25]
        for di in range(K):
            for dj in range(K):
                kp2[:, :, di + 7, dj + 3] = kc[:, :, di * K + dj]
        m_arr = kp2[IH, IW, IDI, IDJ]                    # [wh,dh,du,ww,th,tw]
        m_arr = (m_arr.reshape(NWH // 2, 2, NP, NWW * NT)
                 .transpose(0, 2, 1, 3).reshape(NWH // 2, NP, 2 * NWW * NT))
        in_maps.append({
            "s": np.ascontiguousarray(s_arr),
            "m": np.ascontiguousarray(m_arr),
            "biasr": bias_rep,
        })
    return in_maps


def _unshard(results):
    out = np.empty((B, H, W, C), np.float32)
    for core in range(8):
        b, hh = core // 2, core % 2
        res = np.asarray(results[core]["out"], ml_dtypes.bfloat16)
        r4 = res.reshape(4, C, NWH * NWW // 4, TH, TW)   # [rho,c,sg,th,tw]
        oc = np.empty((NWH, TH, NWW, TW, C), np.float32)
        for rho in range(4):
            # sg = 16*wh + s'  ->  ww = 4*s' + rho
            blk = r4[rho].reshape(C, NWH, NWW // 4, TH, TW)
            oc[:, :, rho::4, :, :] = blk.transpose(1, 3, 2, 4, 0)
        out[b, hh * HS:(hh + 1) * HS] = oc.reshape(HS, W, C)
    return out


def _run(feat, kernel, bias, **run_kwargs):
    nc = _get_nc()
    in_maps = _prep_inputs(feat, kernel, bias)
    res = run_bass_kernel_spmd(nc, in_maps, core_ids=list(range(8)),
                               **run_kwargs)
    return _unshard(res.results), res


def kernel(feat, kernel, bias):
    out, _ = _run(np.asarray(feat, np.float32), np.asarray(kernel, np.float32),
                  np.asarray(bias, np.float32))
    return out


# revision 12
# speedup vs baseline: 1.0336x; 1.0336x over previous
"""Per-pixel predicted 5x5 conv (KPN) on 8 trn2 cores.

Deep-contraction im2col: each 8x4 output tile (one window) is ONE PE matmul
contracting over the 12x8 input patch (96 partitions):

  out[c, t] = sum_p  S[p, c] * M[p, t]
  S[p=(dh,du), c]       = feat[8wh+dh-2, 4ww+du-2, c]   (host-gathered slabs)
  M[p=(dh,du), t=(th,tw)] = kernel[8wh+th, 4ww+tw, (dh-th)*5+(du-tw)]
                            if both tap offsets in [0,5) else 0  (banded)

All 25 taps of 32 output pixels x 32 channels finish in one 32-column
matmul (bf16 in, fp32 PSUM). ACT evacuates PSUM with the bias add fused;
output leaves as bf16. DMA per core: S 6.3MB + M 6.3MB + out 2.1MB.
"""

import sys

for p in ("/opt/pypackages", "/opt/trn_rl_repo"):
    if p not in sys.path:
        sys.path.insert(0, p)

import numpy as np
import ml_dtypes

import concourse.mybir as mybir
from concourse import bacc, tile
from concourse.bass_utils import run_bass_kernel_spmd

B, H, W, C, KK, K = 4, 256, 256, 32, 25, 5
HS = H // 2            # 128 output rows per core
TH, TW = 8, 4          # output tile per window
PH, PU = TH + 4, TW + 4    # input patch dims -> 12*8 = 96 partitions
NP = PH * PU           # 96
NWH, NWW = HS // TH, W // TW   # 16 x 64 = 1024 windows per core
NT = TH * TW           # 32 moving columns per window
SLOTS = 15             # windows of one rho-quadrant per PSUM bank (15*32=480)
NG = (NWH * NWW // 4 + SLOTS - 1) // SLOTS   # 18 evac groups (last partial)
OUTF = (NWH * NWW // 4) * NT  # 8192 free elems per out partition
BF16 = mybir.dt.bfloat16
F32 = mybir.dt.float32

_NC_CACHE = {}


def _build_nc():
    nc = bacc.Bacc(None, target_bir_lowering=False)
    s_d = nc.dram_tensor("s", [NWH // 2, NP, 2 * NWW * C], BF16,
                         kind="ExternalInput")
    m_d = nc.dram_tensor("m", [NWH // 2, NP, 2 * NWW * NT], BF16,
                         kind="ExternalInput")
    bias_d = nc.dram_tensor("biasr", [128, 1], F32, kind="ExternalInput")
    out_d = nc.dram_tensor("out", [128, OUTF], BF16, kind="ExternalOutput")
    GNT = SLOTS * NT                      # 480 out elems per evac group

    with tile.TileContext(nc) as tc:
        with tc.tile_pool(name="const", bufs=1) as cpool, \
             tc.tile_pool(name="sm", bufs=3) as spool, \
             tc.tile_pool(name="osb", bufs=4) as opool, \
             tc.tile_pool(name="psum", bufs=4, space="PSUM") as qpool:
            bias_t = cpool.tile([128, 1], F32, tag="bias")
            nc.sync.dma_start(out=bias_t, in_=bias_d[:, :])

            ps = ob = None
            for wh in range(NWH):
                if wh % 2 == 0:
                    s_t = spool.tile([NP, 2, NWW, C], BF16, tag="s")
                    nc.sync.dma_start(out=s_t, in_=s_d[wh // 2, :, :])
                    m_t = spool.tile([NP, 2, NWW, NT], BF16, tag="m")
                    nc.sync.dma_start(out=m_t, in_=m_d[wh // 2, :, :])
                for ww in range(NWW):
                    w = wh * NWW + ww        # window index
                    rho, sg = w % 4, w // 4
                    g, slot = sg // SLOTS, sg % SLOTS
                    if rho == 0 and slot == 0:
                        ps = qpool.tile([128, GNT], F32, tag="ps")
                    nc.tensor.matmul(
                        ps[32 * rho:32 * rho + 32,
                           slot * NT:(slot + 1) * NT],
                        s_t[:, wh % 2, ww, :],
                        m_t[:, wh % 2, ww, :],
                        start=True, stop=True,
                        tile_position=(0, 32 * rho))
                    if w == NWH * NWW - 1 or (rho == 3 and slot == SLOTS - 1):
                        nf = (slot + 1) * NT
                        if g % 2 == 0:
                            ob = opool.tile([128, 2 * GNT], BF16, tag="ob")
                        nc.scalar.activation(
                            ob[:, (g % 2) * GNT:(g % 2) * GNT + nf],
                            ps[:, :nf],
                            mybir.ActivationFunctionType.Identity,
                            bias=bias_t[:, :], scale=1.0)
                        if g % 2 == 1 or w == NWH * NWW - 1:
                            g0 = g - (g % 2)
                            tot = (g % 2) * GNT + nf
                            nc.gpsimd.dma_start(
                                out=out_d[:, g0 * GNT:g0 * GNT + tot],
                                in_=ob[:, :tot])
    if not nc.is_finalized():
        nc.finalize()
    return nc


def _get_nc():
    if "nc" not in _NC_CACHE:
        _NC_CACHE["nc"] = _build_nc()
    return _NC_CACHE["nc"]


def _prep_inputs(feat, kernel, bias):
    fb = feat.astype(ml_dtypes.bfloat16)
    kb = kernel.astype(ml_dtypes.bfloat16)
    # broadcastable index arrays over [wh, dh, du, ww, th, tw]
    IH = (8 * np.arange(NWH)[:, None, None, None, None, None]
          + np.arange(TH)[None, None, None, None, :, None]).astype(np.int32)
    IW = (4 * np.arange(NWW)[None, None, None, :, None, None]
          + np.arange(TW)[None, None, None, None, None, :]).astype(np.int32)
    IDI = (np.arange(PH)[None, :, None, None, None, None]
           - np.arange(TH)[None, None, None, None, :, None] + 7).astype(np.int32)
    IDJ = (np.arange(PU)[None, None, :, None, None, None]
           - np.arange(TW)[None, None, None, None, None, :] + 3).astype(np.int32)

    bias_rep = np.ascontiguousarray(
        np.tile(bias.astype(np.float32), 4)[:, None])    # [128,1]

    in_maps = []
    for core in range(8):
        b, hh = core // 2, core % 2
        # padded feat rows [-2, 130) x cols [-2, 258)
        fpad = np.zeros((PH + 8 * (NWH - 1), W + 4, C), ml_dtypes.bfloat16)
        r0 = hh * HS - 2
        lo, hi = max(0, -r0), min(132, H - r0)
        fpad[lo:hi, 2:W + 2] = fb[b, r0 + lo:r0 + hi]
        # S[wh, p=(dh,du), ww, c] = fpad[8wh+dh, 4ww+du, c]
        s_arr = fpad[(8 * np.arange(NWH)[:, None, None, None]
                      + np.arange(PH)[None, :, None, None]),
                     (4 * np.arange(NWW)[None, None, None, :]
                      + np.arange(PU)[None, None, :, None])]  # [wh,dh,du,ww,c]
        s_arr = (s_arr.reshape(NWH // 2, 2, NP, NWW * C)
                 .transpose(0, 2, 1, 3).reshape(NWH // 2, NP, 2 * NWW * C))
        # padded tap table for this core
        kp2 = np.zeros((HS, W, PH + TH - 1, PU + TW - 1), ml_dtypes.bfloat16)
        kc = kb[b, hh * HS:(hh + 1) * HS]                # [128, 256, 25]
        for di in range(K):
            for dj in range(K):
                kp2[:, :, di + 7, dj + 3] = kc[:, :, di * K + dj]
        m_arr = kp2[IH, IW, IDI, IDJ]                    # [wh,dh,du,ww,th,tw]
        m_arr = (m_arr.reshape(NWH // 2, 2, NP, NWW * NT)
                 .transpose(0, 2, 1, 3).reshape(NWH // 2, NP, 2 * NWW * NT))
        in_maps.append({
            "s": np.ascontiguousarray(s_arr),
            "m": np.ascontiguousarray(m_arr),
            "biasr": bias_rep,
        })
    return in_maps


def _unshard(results):
    out = np.empty((B, H, W, C), np.float32)
    for core in range(8):
        b, hh = core // 2, core % 2
        res = np.asarray(results[core]["out"], ml_dtypes.bfloat16)
        r4 = res.reshape(4, C, NWH * NWW // 4, TH, TW)   # [rho,c,sg,th,tw]
        oc = np.empty((NWH, TH, NWW, TW, C), np.float32)
        for rho in range(4):
            # sg = 16*wh + s'  ->  ww = 4*s' + rho
            blk = r4[rho].reshape(C, NWH, NWW // 4, TH, TW)
            oc[:, :, rho::4, :, :] = blk.transpose(1, 3, 2, 4, 0)
        out[b, hh * HS:(hh + 1) * HS] = oc.reshape(HS, W, C)
    return out


def _run(feat, kernel, bias, **run_kwargs):
    nc = _get_nc()
    in_maps = _prep_inputs(feat, kernel, bias)
    res = run_bass_kernel_spmd(nc, in_maps, core_ids=list(range(8)),
                               **run_kwargs)
    return _unshard(res.results), res


def kernel(feat, kernel, bias):
    out, _ = _run(np.asarray(feat, np.float32), np.asarray(kernel, np.float32),
                  np.asarray(bias, np.float32))
    return out
